# revision 58
# baseline (speedup 1.0000x reference)
"""BehaviorMoE Trainium2 kernel (8 NeuronCores, SPMD data-parallel over sorted tokens).

Contract: kernel(**inputs) takes FULL inputs as returned by setup_inputs() and
returns the FULL [8192, 1024] float32 output.

Strategy (v14):
  - Host: sort tokens by behavior id. Each behavior t in 1..4 owns two cores;
    each core gets M=896 tokens (7x128 tiles) of that single behavior, padded
    with masked b==0 filler. Leftover b==0 tokens (out = x + beta) never touch
    the device. The residual add (x +) happens on host during the scatter, so
    the device returns LN(moe(x)) in bf16 (halves output DMA, drops xtok).
  - Device (identical SPMD program, per-core data):
      All matmuls in fp16 — fp16 runs at bf16 PE rate (216ns/instr floor
      regardless of width) with better mantissa; one fp16 x buffer serves
      both the expert matmuls (stationary) and the gate logits (moving).
      The whole opening feed lives on the Sync queue in strict order
      (cross-queue DMA service is unordered and the scheduler hoists ready
      descriptors): interleaved xh/w00 k-chunk pairs pace the opening
      wave, then mask/ball/w01/w1; wg rides Scalar; w2/w3 ride GpSimd
      behind a WAW corner-write on w11 so their transfers can't steal
      opening bandwidth (the issuing engine blocks on unresolved deps, so
      they sit on the otherwise-idle queue). 32 junk warmup matmuls keep
      the PE busy through its p-state ramp while the cold DMA path (which
      has its own clock ramp) delivers the first chunks.
      PE stream order: e0c0 wave over all 7 tiles + gate logits fused in
      (glc0/glc1 share one PSUM bank at partition offsets 0/32), last k
      tile-major so the copy-outs stagger across Scalar/Vector; first
      e0c1 group; glp transposes; then the softmax core runs on V/S while
      the PE streams the remaining e0c1 groups; deferred gate-scales;
      e1 with paired gates-transposes (two tiles per PE transpose at
      partition offsets 0/32, ball duplicated at both bases) and bias
      combine matmuls (pre-scaled by gates so the join is a plain add on
      GpSimd/Vector); fused e2+e3 with per-tile LN tails deferred by one
      tile. LN normalize writes bf16; the last tile's normalize/DMA is
      split across warm queues to shorten the exposed tail.
  - Host: scatter per-core outputs back to original token order, adding x.
"""

import os
import sys

import numpy as np
import ml_dtypes

for _p in ("/opt/trn_rl_repo", "/root/.axon_site/_ro/trn_rl_repo"):
    if os.path.isdir(_p) and _p not in sys.path:
        sys.path.append(_p)

from contextlib import ExitStack

from concourse import bacc, bass, masks, mybir, tile
from concourse.bass_utils import run_bass_kernel_spmd

F32 = mybir.dt.float32
F32R = mybir.dt.float32r
BF16 = mybir.dt.bfloat16
F16 = mybir.dt.float16
F8 = mybir.dt.float8e4
AX = mybir.AxisListType
ALU = mybir.AluOpType
ACTF = mybir.ActivationFunctionType

D = 1024            # model dim
N = 8192            # tokens
NB = 4              # behaviors
NESH = 3            # shared experts
NE = 4              # experts per behavior (3 shared + 1 specific)
EPS = 1e-5
NCORES = 8
M = 896             # tokens per core (7 tiles of 128)
KT = D // 128       # k tiles (contraction)
KC = 6              # fp16 k tiles; the last 2 ride one fp8 DoubleRow matmul
IT = M // 128       # token tiles per core
FH = 512            # feature half-tile (psum bank width in f32)
C1 = M - FH         # second logit token chunk (384)


def _build_program(trivial_affine: bool) -> bass.Bass:
    nc = bacc.Bacc()

    xh_d = nc.declare_dram_parameter("xh", [128, KT * M], F16, isOutput=False)
    x8_d = nc.declare_dram_parameter("x8", [128, 2 * M], F8, isOutput=False)
    wt_d = nc.declare_dram_parameter("wt", [NE, 2, 128, KC * FH], F16, isOutput=False)
    w8_d = nc.declare_dram_parameter("w8", [NE, 2, 128, 2 * FH], F8, isOutput=False)
    wg_d = nc.declare_dram_parameter("wg", [128, KT * NE], F16, isOutput=False)
    ball_d = nc.declare_dram_parameter("ball", [NE, D], BF16, isOutput=False)
    mask_d = nc.declare_dram_parameter("mask", [128, IT], F32, isOutput=False)
    if not trivial_affine:
        gam_d = nc.declare_dram_parameter("gam", [128, D], F32, isOutput=False)
        bet_d = nc.declare_dram_parameter("bet", [128, D], F32, isOutput=False)
    out_d = nc.declare_dram_parameter("out", [M, D], BF16, isOutput=True)

    with tile.TileContext(nc) as tc, ExitStack() as ctx:
        const = ctx.enter_context(tc.tile_pool(name="const", bufs=1))
        wpool = ctx.enter_context(tc.tile_pool(name="w", bufs=4 * NE))
        selp = ctx.enter_context(tc.tile_pool(name="sel", bufs=2 * IT))
        outp = ctx.enter_context(tc.tile_pool(name="outp", bufs=3))
        scrp = ctx.enter_context(tc.tile_pool(name="scr", bufs=3))
        biasp = ctx.enter_context(tc.tile_pool(name="bias", bufs=3))
        gatep = ctx.enter_context(tc.tile_pool(name="gate", bufs=IT))
        gtsp = ctx.enter_context(tc.tile_pool(name="gts", bufs=2 * IT))
        smallp = ctx.enter_context(tc.tile_pool(name="small", bufs=16))
        zpool = ctx.enter_context(tc.tile_pool(name="z", bufs=6, space="PSUM"))
        pspool = ctx.enter_context(tc.tile_pool(name="ps", bufs=2, space="PSUM"))

        # ---- PE warmup fodder: junk tile memset by GpSimd, then 16 tiny
        # matmuls keep the PE busy through its p-state ramp while DMAs land
        junk = const.tile([128, 256], F16, tag="junk")
        nc.gpsimd.memset(junk[:], 0)
        eps_sb = const.tile([128, 1], F32, tag="eps")
        nc.gpsimd.memset(eps_sb[:], EPS)

        # ---- DMA issue. Cross-queue transfer service order is not
        # priority-ordered (and the scheduler hoists ready descriptors), so
        # the whole critical opening stream lives on the Sync queue in
        # strict order: interleaved xh/w00 k-chunk pairs pace the e0c0
        # wave, then small tensors, then w01/w1 in deadline order. ----
        xh = const.tile([128, KT * M], F16, tag="xh")
        x8 = const.tile([128, 2, M], F8, tag="x8")
        w_sb = {}
        w8_sb = {}
        for e in range(NE):
            for c in (0, 1):
                w_sb[(e, c)] = wpool.tile(
                    [128, KC * FH], F16, tag="w", name=f"w{e}{c}"
                )
                w8_sb[(e, c)] = wpool.tile(
                    [128, 2, FH], F8, tag="w8", name=f"w8{e}{c}"
                )
        # wg rides the Scalar queue (tiny; must not burn a cold Sync slot
        # ahead of the first xh chunk)
        wg_sb = const.tile([128, KT * NE], F16, tag="wg")
        nc.scalar.dma_start(wg_sb[:], wg_d[:])
        # first chunk split so the opening matmul waits on ~157KB of cold
        # DMA instead of 357KB (the wave is PE-bound, so an earlier start
        # shifts the whole stream earlier)
        nc.sync.dma_start(xh[:, 0:128], xh_d[:, 0:128])
        nc.sync.dma_start(w_sb[(0, 0)][:, 0:FH], wt_d[0, 0, :, 0:FH])
        nc.sync.dma_start(xh[:, 128:M], xh_d[:, 128:M])
        for j in range(1, KC):
            nc.sync.dma_start(
                xh[:, j * M:(j + 1) * M], xh_d[:, j * M:(j + 1) * M]
            )
            nc.sync.dma_start(
                w_sb[(0, 0)][:, j * FH:(j + 1) * FH],
                wt_d[0, 0, :, j * FH:(j + 1) * FH],
            )
        nc.sync.dma_start(xh[:, KC * M:], xh_d[:, KC * M:])
        nc.sync.dma_start(x8[:], x8_d[:])
        nc.sync.dma_start(w8_sb[(0, 0)][:], w8_d[0, 0])
        mask_sb = const.tile([128, IT], F32, tag="mask")
        nc.sync.dma_start(mask_sb[:], mask_d[:])
        # ball duplicated at partition bases 0 and 32 (the bias matmuls'
        # stationary gatesT alternates between those bases)
        ball_sb = const.tile([32 + NE, D], BF16, tag="ball")
        nc.sync.dma_start(ball_sb[0:NE, :], ball_d[:])
        nc.sync.dma_start(ball_sb[32:32 + NE, :], ball_d[:])
        if not trivial_affine:
            gam_sb = const.tile([128, D], F32, tag="gam")
            nc.sync.dma_start(gam_sb[:], gam_d[:])
            bet_sb = const.tile([128, D], F32, tag="bet")
            nc.sync.dma_start(bet_sb[:], bet_d[:])
        nc.sync.dma_start(w_sb[(0, 1)][:], wt_d[0, 1])
        nc.sync.dma_start(w8_sb[(0, 1)][:], w8_d[0, 1])
        nc.sync.dma_start(w_sb[(1, 0)][:], wt_d[1, 0])
        nc.sync.dma_start(w8_sb[(1, 0)][:], w8_d[1, 0])
        nc.sync.dma_start(w_sb[(1, 1)][:], wt_d[1, 1])
        nc.sync.dma_start(w8_sb[(1, 1)][:], w8_d[1, 1])

        # ---- PE warmup (junk matmuls; output never read). Enough of them
        # to keep the PE continuously busy until the first data lands, so
        # the p-state ramp isn't reset by an idle gap before the wave. ----
        warm = pspool.tile([128, FH], F32, tag="ps", name="warm")
        for r in range(26):
            nc.tensor.matmul(
                warm[:, 0:128], junk[:, 0:128], junk[:, 128:256],
                start=True, stop=True, skip_group_check=True,
            )

        # ---- identity/constant prep (overlaps DMA wait) ----
        identity = const.tile([128, 128], F32, tag="ident")
        masks.make_identity(nc, identity[:])
        identB = const.tile([128, 128], BF16, tag="identB")
        nc.vector.tensor_copy(identB[:], identity[:])
        identR = const.tile([NE, NE], F32R, tag="identR")
        nc.vector.tensor_copy(identR[:], identity[0:NE, 0:NE])

        # Late weights on the GpSimd queue (idle until the bias joins),
        # each gated behind w11 via a WAW corner-write — a true data
        # dependency the scheduler cannot hoist — so their transfers
        # cannot steal opening-phase HBM bandwidth. (The issuing engine
        # blocks until the dependency resolves, which is why these are
        # not on Sync/Scalar.)
        for e in (2, 3):
            for c in (0, 1):
                nc.gpsimd.dma_start(
                    w_sb[(e, c)][:, 0:4], w_sb[(1, 1)][:, KC * FH - 4:]
                )
                nc.gpsimd.dma_start(w_sb[(e, c)][:], wt_d[e, c])
                nc.gpsimd.dma_start(
                    w8_sb[(e, c)][:, 0, 0:8], w8_sb[(1, 1)][:, 0, 0:8]
                )
                nc.gpsimd.dma_start(w8_sb[(e, c)][:], w8_d[e, c])

        # ---- accumulators (ping-pong; in-place DVE ops fault) ----
        selA = [selp.tile([128, D], BF16, tag="sel", name=f"selA{i}")
                for i in range(IT)]
        selB = [selp.tile([128, D], BF16, tag="sel", name=f"selB{i}")
                for i in range(IT)]

        def isl(i):
            return slice(i * 128, (i + 1) * 128)

        def csl(c):
            return slice(c * FH, (c + 1) * FH)

        def expert_mms(zt, e, c, i):
            for k in range(KC):
                nc.tensor.matmul(
                    zt[:], xh[:, k * M + i * 128:k * M + (i + 1) * 128],
                    w_sb[(e, c)][:, k * FH:(k + 1) * FH],
                    start=(k == 0), stop=False,
                )
            # last 2 k-tiles in one fp8 DoubleRow matmul (2x MACs/instr)
            nc.tensor.matmul(
                zt[:], x8[:, :, i * 128:(i + 1) * 128], w8_sb[(e, c)][:],
                start=False, stop=True,
                perf_mode=mybir.MatmulPerfMode.DoubleRow,
            )

        def expert_group(e, c, i):
            zt = zpool.tile([128, FH], F32, tag="z")
            expert_mms(zt, e, c, i)
            return zt

        # ---- e0/c0 wave for ALL 7 tiles + gate logits, k-outer for k=0..6
        # (paced by the k-chunk DMAs; the glc matmuls are free PE work in
        # the DMA-bound window), then tile-major k=7 so the stops stagger
        # and the PSUM copy-outs drain across Scalar/Vector. glc0/glc1
        # share one PSUM bank at partition offsets 0 and 32. ----
        zt_e0c0 = [zpool.tile([128, FH], F32, tag="z", name=f"zw{i}")
                   for i in range(6)]
        zt_e0c0.append(pspool.tile([128, FH], F32, tag="ps", name="zw6"))
        glc = pspool.tile([128, FH], F32, tag="ps", name="glc")

        def glc_mms(k):
            nc.tensor.matmul(
                glc[0:NE, :], wg_sb[:, k * NE:(k + 1) * NE],
                xh[:, k * M:k * M + FH],
                start=(k == 0), stop=(k == KT - 1), skip_group_check=True,
            )
            nc.tensor.matmul(
                glc[32:32 + NE, 0:C1], wg_sb[:, k * NE:(k + 1) * NE],
                xh[:, k * M + FH:(k + 1) * M],
                start=(k == 0), stop=(k == KT - 1), skip_group_check=True,
            )

        for k in range(KC):
            for i in range(IT):
                nc.tensor.matmul(
                    zt_e0c0[i][:], xh[:, k * M + isl(i).start:k * M + isl(i).stop],
                    w_sb[(0, 0)][:, k * FH:(k + 1) * FH],
                    start=(k == 0), stop=False,
                )
            glc_mms(k)
        glc_mms(KC)
        glc_mms(KC + 1)
        for i in range(IT):
            # finishing fp8 DoubleRow matmul, then the copy-out with NO
            # gate dependency; copies alternate engines (GpSimd cannot
            # read PSUM, so spread over Scalar/Vector)
            nc.tensor.matmul(
                zt_e0c0[i][:], x8[:, :, i * 128:(i + 1) * 128],
                w8_sb[(0, 0)][:],
                start=False, stop=True,
                perf_mode=mybir.MatmulPerfMode.DoubleRow,
            )
            if i % 2 == 0:
                nc.scalar.copy(selA[i][:, csl(0)], zt_e0c0[i][:])
            else:
                nc.vector.tensor_copy(selA[i][:, csl(0)], zt_e0c0[i][:])

        glT_sb = const.tile([NE, M], F32R, tag="glT")
        nc.vector.tensor_copy(glT_sb[:, 0:FH], glc[0:NE, :])
        nc.vector.tensor_copy(glT_sb[:, FH:M], glc[32:32 + NE, 0:C1])

        # ---- e0/c1 group for tile 0 only (covers the glT-copy latency so
        # the glp transposes never stall the PE) ----
        e0c1_zts = {}
        e0c1_zts[0] = expert_group(0, 1, 0)

        # ---- transpose logits to token-major ----
        glp_all = zpool.tile([128, FH], F32, tag="z", name="glp")
        for i in range(IT):
            nc.tensor.matmul(
                glp_all[:, NE * i:NE * (i + 1)], glT_sb[:, isl(i)], identR[:],
                start=True, stop=True, skip_group_check=True,
            )
        glps = const.tile([128, 32], F32, tag="glps")
        nc.vector.tensor_copy(glps[:, 0:NE * IT], glp_all[:, 0:NE * IT])

        # ---- masked softmax core for ALL tiles (runs on V/S while the PE
        # streams the rest of e0/c1; gate-scales are deferred so they don't
        # delay the e0/c1 PSUM copy-outs) ----
        gates_t = []
        rm_t = []
        for i in range(IT):
            negmax = smallp.tile([128, 16], F32, tag="s1", name="negmax")[:, 0:1]
            nc.vector.tensor_reduce(
                negmax[:], glps[:, NE * i:NE * (i + 1)],
                axis=AX.X, op=ALU.max, negate=True,
            )
            exps = smallp.tile([128, 16], F32, tag="s4", name="exps")[:, 0:NE]
            expsum = smallp.tile([128, 16], F32, tag="s1", name="expsum")[:, 0:1]
            nc.scalar.activation(
                exps[:], glps[:, NE * i:NE * (i + 1)], ACTF.Exp,
                bias=negmax[:], scale=1.0, accum_out=expsum[:],
            )
            rinv = smallp.tile([128, 16], F32, tag="s1", name="rinv")[:, 0:1]
            nc.vector.reciprocal(rinv[:], expsum[:])
            rm = smallp.tile([128, 16], F32, tag="rm", name=f"rm{i}")[:, 0:1]
            nc.vector.tensor_mul(rm[:], rinv[:], mask_sb[:, i:i + 1])
            rm_t.append(rm)
            gates = gatep.tile([128, 16], F32, tag="g", name="gates")[:, 0:NE]
            nc.vector.tensor_scalar_mul(gates[:], exps[:], rm[:])
            gates_t.append(gates)

        # gates in bf16 for the PE transpose. Using gates (= exps * rm)
        # instead of raw exps makes the bias combine come out pre-scaled,
        # so its join is a plain add. Two tiles pack into one transpose at
        # column offsets 0/32 so one PE transpose serves a pair (stationary
        # base partitions 0 and 32 are both legal for the 4-row bias
        # matmuls).
        NPAIR = (IT + 1) // 2
        gB2 = []
        for p in range(NPAIR):
            g2 = gtsp.tile([128, 64], BF16, tag="eB", name=f"eB{p}")
            nc.gpsimd.memset(g2[:], 0)
            gB2.append(g2)
        for i in range(IT):
            nc.scalar.copy(
                gB2[i // 2][:, 32 * (i % 2):32 * (i % 2) + NE], gates_t[i][:]
            )

        # ---- e0/c1 for the remaining tiles (copies spread V/S) ----
        for i in range(1, IT):
            e0c1_zts[i] = expert_group(0, 1, i)
        for i in range(IT):
            if i % 2 == 0:
                nc.vector.tensor_copy(selA[i][:, csl(1)], e0c1_zts[i][:])
            else:
                nc.scalar.copy(selA[i][:, csl(1)], e0c1_zts[i][:])

        # ---- gate-scale of the raw e0 accumulator, all on Vector: Scalar
        # must stay free to drain the expsT/bias PSUM copies that pace the
        # e1 phase's 2-buffer bias-bank recycling ----
        for i in range(IT):
            nc.vector.tensor_scalar_mul(
                selB[i][:, csl(0)], selA[i][:, csl(0)], gates_t[i][:, 0:1]
            )
            nc.vector.tensor_scalar_mul(
                selB[i][:, csl(1)], selA[i][:, csl(1)], gates_t[i][:, 0:1]
            )

        # ---- e1 (+ exps^T transpose and bias combine join per (c, tile)).
        # The tiny transpose/bias matmuls for tile i are deferred until
        # after tile i+1's expert group, so a late gB2/expsT copy can
        # never head-of-line-block the Tensor queue. ----
        expsT_t = []

        def e1_smalls(c, i):
            if c == 0 and i % 2 == 0:
                gtp = pspool.tile([128, FH], F32, tag="ps", name=f"gtp{i}")
                nc.tensor.matmul(
                    gtp[0:36, 0:128], gB2[i // 2][:, 0:36], identB[:],
                    start=True, stop=True,
                )
                expsT = gtsp.tile([36, 128], BF16, tag="eT", name=f"eT{i}")
                nc.scalar.copy(expsT[:], gtp[0:36, 0:128])
                expsT_t.append(expsT)
            bp = pspool.tile([128, FH], F32, tag="ps", name=f"bp{i}{c}")
            ofs = 32 * (i % 2)
            nc.tensor.matmul(
                bp[:], expsT_t[i // 2][ofs:ofs + NE, :],
                ball_sb[ofs:ofs + NE, csl(c)],
                start=True, stop=True,
            )
            bias_sb = biasp.tile([128, FH], F32, tag="bias")
            nc.scalar.copy(bias_sb[:], bp[:])
            # bias join (bias already gate-scaled): SBUF-only tensor add
            # on GpSimd, keeping Vector free for the STT joins that pace
            # the PSUM bank recycling
            nc.gpsimd.tensor_add(
                selB[i][:, csl(c)], bias_sb[:], selA[i][:, csl(c)]
            )

        pending_small = None
        for c in (0, 1):
            for i in range(IT):
                zt = expert_group(1, c, i)
                nc.vector.scalar_tensor_tensor(
                    selA[i][:, csl(c)], zt[:], gates_t[i][:, 1:2],
                    selB[i][:, csl(c)], op0=ALU.mult, op1=ALU.add,
                )
                if pending_small is not None:
                    e1_smalls(*pending_small)
                pending_small = (c, i)
        e1_smalls(*pending_small)

        # ---- fused e2+e3 passes with per-tile LN tail ----
        # Each tile's post-bn LN chain is deferred until after the NEXT
        # tile's c0 STTs so the Vector stream frees PSUM banks back-to-back.
        def ln_tail(i, bn6):
            selF = selB[i]
            mv = smallp.tile([128, 16], F32, tag="mv", name="mv")[:, 0:2]
            nc.vector.bn_aggr(mv[:], bn6[:])
            # fused 1/sqrt(var+eps) in one Scalar op (shortens the exposed
            # last-tile chain by a cross-engine hop); var >= 0 so the abs
            # is a no-op
            ri = smallp.tile([128, 16], F32, tag="s1", name="ri")[:, 0:1]
            nc.scalar.activation(
                ri[:], mv[:, 1:2], ACTF.Abs_reciprocal_sqrt, bias=eps_sb[:]
            )
            mbi = smallp.tile([128, 16], F32, tag="s1", name="mbi")[:, 0:1]
            nc.vector.scalar_tensor_tensor(
                mbi[:], mv[:, 0:1], -1.0, ri[:], op0=ALU.mult, op1=ALU.mult
            )
            # ln = sel*rstd + mb; output directly in bf16 (residual is added
            # on host). Last tile splits work across engines/queues to
            # shorten the exposed tail.
            outt = outp.tile([128, D], BF16, tag="out")
            last = i == IT - 1
            if trivial_affine:
                if last:
                    qs = slice(FH, 1024)
                    nc.vector.tensor_scalar(
                        outt[:, qs], selF[:, qs], ri[:], mbi[:],
                        op0=ALU.mult, op1=ALU.add,
                    )
                    nc.sync.dma_start(out_d[isl(i), qs], outt[:, qs])
                    qs = slice(0, FH)
                    nc.scalar.activation(
                        outt[:, qs], selF[:, qs], ACTF.Identity,
                        bias=mbi[:], scale=ri[:],
                    )
                    nc.scalar.dma_start(out_d[isl(i), qs], outt[:, qs])
                else:
                    nc.scalar.activation(
                        outt[:, csl(0)], selF[:, csl(0)], ACTF.Identity,
                        bias=mbi[:], scale=ri[:],
                    )
                    nc.sync.dma_start(out_d[isl(i), csl(0)], outt[:, csl(0)])
                    nc.vector.tensor_scalar(
                        outt[:, csl(1)], selF[:, csl(1)], ri[:], mbi[:],
                        op0=ALU.mult, op1=ALU.add,
                    )
                    nc.scalar.dma_start(out_d[isl(i), csl(1)], outt[:, csl(1)])
            else:
                for c in (0, 1):
                    lnb = scrp.tile([128, FH], F32, tag="scr")
                    nc.scalar.activation(
                        lnb[:], selF[:, csl(c)], ACTF.Identity,
                        bias=mbi[:], scale=ri[:],
                    )
                    lng = scrp.tile([128, FH], F32, tag="scr")
                    nc.vector.tensor_mul(lng[:], lnb[:], gam_sb[:, csl(c)])
                    nc.vector.tensor_add(outt[:, csl(c)], lng[:], bet_sb[:, csl(c)])
                    dma_eng = nc.scalar if c == 1 else nc.sync
                    dma_eng.dma_start(out_d[isl(i), csl(c)], outt[:, csl(c)])

        pending = None
        for i in range(IT):
            last = i == IT - 1
            bn6 = smallp.tile([128, 32], F32, tag="bn6", name="bn6")
            bn6 = bn6[:, 0:18] if last else bn6[:, 0:12]
            for c in (0, 1):
                zt2 = expert_group(2, c, i)
                nc.vector.scalar_tensor_tensor(
                    selA[i][:, csl(c)], zt2[:], gates_t[i][:, 2:3],
                    selB[i][:, csl(c)], op0=ALU.mult, op1=ALU.add,
                )
                if last and c == 1:
                    # final accumulation in two half-width groups on
                    # separate banks: the first half's join+stats hide
                    # under the second half's matmuls, shortening the
                    # exposed tail chain
                    for h in (0, 1):
                        zt3 = zpool.tile([128, FH], F32, tag="z")
                        for k in range(KC):
                            nc.tensor.matmul(
                                zt3[:, 0:256],
                                xh[:, k * M + isl(i).start:k * M + isl(i).stop],
                                w_sb[(3, 1)][:, k * FH + h * 256:
                                             k * FH + h * 256 + 256],
                                start=(k == 0), stop=False,
                            )
                        nc.tensor.matmul(
                            zt3[:, 0:256], x8[:, :, i * 128:(i + 1) * 128],
                            w8_sb[(3, 1)][:, :, h * 256:(h + 1) * 256],
                            start=False, stop=True,
                            perf_mode=mybir.MatmulPerfMode.DoubleRow,
                        )
                        fs = slice(FH + h * 256, FH + (h + 1) * 256)
                        nc.vector.scalar_tensor_tensor(
                            selB[i][:, fs], zt3[:, 0:256], gates_t[i][:, 3:4],
                            selA[i][:, fs], op0=ALU.mult, op1=ALU.add,
                        )
                        nc.vector.bn_stats(
                            bn6[:, 6 + 6 * h:12 + 6 * h], selB[i][:, fs]
                        )
                else:
                    zt3 = expert_group(3, c, i)
                    nc.vector.scalar_tensor_tensor(
                        selB[i][:, csl(c)], zt3[:], gates_t[i][:, 3:4],
                        selA[i][:, csl(c)], op0=ALU.mult, op1=ALU.add,
                    )
                    # bn stats for each half as soon as the accumulator
                    # is final
                    nc.vector.bn_stats(
                        bn6[:, 6 * c:6 * (c + 1)], selB[i][:, csl(c)]
                    )
                if pending is not None and c == 0:
                    ln_tail(*pending)
                    pending = None
            pending = (i, bn6)
        ln_tail(*pending)

    nc.finalize()
    return nc


_PROGRAM_CACHE: dict = {}


def _get_program(trivial_affine: bool) -> bass.Bass:
    key = trivial_affine
    if key not in _PROGRAM_CACHE:
        _PROGRAM_CACHE[key] = _build_program(trivial_affine)
    return _PROGRAM_CACHE[key]


def _pack_tokens(b: np.ndarray):
    """Two cores per behavior t in 1..4, M=896 tokens each, padded with masked
    b==0 filler. Returns (per-core (idx, mask, t) list, leftover b==0 idx)."""
    idx0 = np.flatnonzero(b == 0)
    p0 = 0
    cores = []
    for t in range(1, NB + 1):
        idxs = np.flatnonzero(b == t)
        if len(idxs) > 2 * M:
            raise RuntimeError(
                f"behavior {t} has {len(idxs)} tokens > capacity {2 * M}"
            )
        for s in (0, M):
            part = idxs[s:s + M]
            need = M - len(part)
            fill = idx0[p0:p0 + need]
            p0 += need
            if len(fill) != need:
                raise RuntimeError("not enough b==0 filler tokens for packing")
            idx = np.concatenate([part.astype(np.int64), fill.astype(np.int64)])
            msk = np.zeros((M,), np.float32)
            msk[:len(part)] = 1.0
            cores.append((idx, msk, t))
    return cores, idx0[p0:]


def _behavior_tensors(W_sh, b_sh, W_sp, b_sp, w_gates):
    per_t = {}
    W_sh_flat = W_sh.reshape(NESH * D, D)
    for t in range(1, NB + 1):
        Wall = np.concatenate([W_sh_flat, W_sp[t - 1:t].reshape(D, D)], axis=0)
        wT = np.ascontiguousarray(Wall.T)                      # [D, NE*D]
        # [e, c, p, k*FH + f] = wT[128k + p, e*D + c*FH + f]; fp16 part
        # covers k tiles 0..KC-1, the last 2 k tiles go to fp8 DoubleRow
        w5 = wT.reshape(KT, 128, NE, 2, FH)
        wt_h = np.ascontiguousarray(
            w5[:KC].transpose(2, 3, 1, 0, 4)
            .reshape(NE, 2, 128, KC * FH).astype(np.float16)
        )
        w8_h = np.ascontiguousarray(
            w5[KC:].transpose(2, 3, 1, 0, 4)
            .reshape(NE, 2, 128, 2 * FH).astype(ml_dtypes.float8_e4m3)
        )
        # [p, k*NE + e] = w_gates[t-1][128k + p, e]
        wg_h = np.ascontiguousarray(
            w_gates[t - 1].reshape(KT, 128, NE).transpose(1, 0, 2)
            .reshape(128, KT * NE).astype(np.float16)
        )
        ball_h = np.stack([b_sh[0], b_sh[1], b_sh[2], b_sp[t - 1]], axis=0)
        per_t[t] = (wt_h, w8_h, wg_h,
                    np.ascontiguousarray(ball_h).astype(ml_dtypes.bfloat16))
    return per_t


def _prepare(x, b_seq, W_sh, b_sh, W_sp, b_sp, w_gates, gamma, beta):
    x = np.ascontiguousarray(np.asarray(x, dtype=np.float32))
    b = np.asarray(b_seq).astype(np.int64).ravel()
    W_sh = np.asarray(W_sh, dtype=np.float32)
    b_sh = np.asarray(b_sh, dtype=np.float32)
    W_sp = np.asarray(W_sp, dtype=np.float32)
    b_sp = np.asarray(b_sp, dtype=np.float32)
    w_gates = np.asarray(w_gates, dtype=np.float32)
    gamma = np.asarray(gamma, dtype=np.float32)
    beta = np.asarray(beta, dtype=np.float32)
    assert x.shape == (N, D) and b.shape == (N,)

    trivial = bool(np.all(gamma == 1.0) and np.all(beta == 0.0))
    cores, leftover = _pack_tokens(b)
    per_t = _behavior_tensors(W_sh, b_sh, W_sp, b_sp, w_gates)

    in_maps = []
    for idx, msk, t in cores:
        wt_h, w8_h, wg_h, ball_h = per_t[t]
        xc = np.ascontiguousarray(x[idx])                      # [M, D]
        # [p, k*M + m] = x[m, 128k + p]
        xT5 = xc.T.reshape(KT, 128, M)
        xh_h = np.ascontiguousarray(
            xT5.transpose(1, 0, 2).reshape(128, KT * M).astype(np.float16)
        )
        # fp8 copy of the last 2 k tiles, [p, j*M + m] = x[m, 128*(KC+j)+p]
        x8_h = np.ascontiguousarray(
            xT5[KC:].transpose(1, 0, 2)
            .reshape(128, 2 * M).astype(ml_dtypes.float8_e4m3)
        )
        m = {
            "xh": xh_h,
            "x8": x8_h,
            "wt": wt_h,
            "w8": w8_h,
            "wg": wg_h,
            "ball": ball_h,
            "mask": np.ascontiguousarray(msk.reshape(IT, 128).T),
        }
        if not trivial:
            m["gam"] = np.ascontiguousarray(np.broadcast_to(gamma, (128, D)))
            m["bet"] = np.ascontiguousarray(np.broadcast_to(beta, (128, D)))
        in_maps.append(m)
    return trivial, cores, leftover, in_maps, x, beta


def kernel_with_results(trace: bool = False, **inputs):
    trivial, cores, leftover, in_maps, x, beta = _prepare(**inputs)
    nc = _get_program(trivial)
    res = run_bass_kernel_spmd(
        nc, in_maps, list(range(NCORES)), trace=trace
    )
    out = np.empty((N, D), np.float32)
    for c, (idx, _msk, _t) in enumerate(cores):
        # device returns ln (bf16); residual add happens here
        out[idx] = x[idx] + np.asarray(res.results[c]["out"], dtype=np.float32)
    if len(leftover):
        out[leftover] = x[leftover] + beta[None, :]
    return out, res


def kernel(**inputs) -> np.ndarray:
    out, _ = kernel_with_results(trace=False, **inputs)
    return out
